# revision 19
# baseline (speedup 1.0000x reference)
"""NeighborDiscriminator kernel for 8x Trainium2 NeuronCores.

Math (reference): augmented-L2 kNN search, k=10, over n=100000 database rows,
B=1024 queries, d=512, followed by max over the k neighbors of
act_i = w_i - ||x_i - q||.

Selection key per (query q, candidate i):
    d2aug = ||q||^2 - 2 q.x_i + ||x_i||^2 + (max(w) - w_i)
Per-query-constant terms don't change the per-query ordering, so the device
ranks by  A = 2 q.x_i + (w_i - ||x_i||^2 - OFF)  (descending A == ascending
d2aug; OFF is a global constant centering the aug term).

Distribution: X / w sharded over 8 cores along n (12500 rows each), queries
replicated.

Device pipeline per core (bf16 was measured fastest on this part; fp8
DoubleRow and plain fp8 are both slower per matmul on real HW):
  - X^T resident in SBUF as bf16 (100KB/partition), queries bf16.
  - scores A = [128q x 500c] in PSUM: 4 plain bf16 matmuls + 1 fp8 rank-1
    that adds the centered aug row (fp8 quantization ~1.4 abs rms, well
    below the ~2-sigma displacement the host margin absorbs).
  - ACT evicts PSUM -> SBUF fp16 into a per-query row h0[128, 12504]
    (a tensor_max reading both PSUM halves is illegal: one PSUM port).
  - DVE halves h0 3x with tensor_max (fp16 2-byte => 2x DVE mode) to 1563
    window maxima; one Max + MaxIndex gives the top-8 windows per query.
Each window covers 8 candidates: h0 columns i + 1563a (a<8) == shard rows.
Host expands the 8 winning windows (64 cand/core, 512/query) and re-ranks
exactly (fp32 cut to 32, fp64 top-k, then max_k(w - dist)).  The top-8
windows provably contain every top-8-by-device-score candidate of the core;
a true top-10 member is missed only if >=8 distinct same-core candidates
outscore it under ~1.5 abs device noise (P ~ 1e-8) or >=9 of the true
top-10 land in one core (P ~ 5e-7).  Verified end-to-end on the reference
seed by hostcheck.py.
"""

import numpy as np
import ml_dtypes

import concourse.bacc as bacc
import concourse.mybir as mybir
from concourse.tile import TileContext
from concourse.bass_utils import run_bass_kernel_spmd

B = 1024            # queries
N_TOTAL = 100000    # database rows
D = 512             # feature dim
M = 8               # cores
NS = N_TOTAL // M   # 12500 rows per core
CT = 500            # candidate tile width (PSUM bank = 512 fp32)
HCT = CT // 2       # 250: eviction pairs column s with s+250
KC = D // 128       # 4 contraction chunks
TOP = 8             # top-8 windows per query per core (DVE max8)
WEXP = 8            # candidates per window (1 evict-halve + 2 cascade halves)

FIN = ml_dtypes.bfloat16

_cached_nc = None


def _h0w(ns):
    """fp16 evicted-row width: ns padded to a multiple of WEXP."""
    return ((ns + WEXP - 1) // WEXP) * WEXP


def _build(b=B, ns=NS):
    qt = b // 128
    nt = ns // CT
    h0w = _h0w(ns)          # 12504 for ns=12500
    win = h0w // WEXP       # 1563

    nc = bacc.Bacc(
        "TRN2",
        target_bir_lowering=False,
        debug=False,
        enable_asserts=False,
        num_devices=M,
    )
    fin = mybir.dt.bfloat16
    f16 = mybir.dt.float16
    f8 = mybir.dt.float8e4

    q8 = nc.dram_tensor("q8", [D, b], fin, kind="ExternalInput")
    x8 = nc.dram_tensor("x8", [D, ns], fin, kind="ExternalInput")
    aug8 = nc.dram_tensor("aug8", [1, 1, ns], f8, kind="ExternalInput")
    ones8 = nc.dram_tensor("ones8", [1, 1, 128], f8, kind="ExternalInput")
    idxs = nc.dram_tensor("idxs", [b, TOP], mybir.dt.uint16, kind="ExternalOutput")

    with TileContext(nc) as tc:
        with (
            tc.tile_pool(name="const", bufs=1) as cpool,
            tc.tile_pool(name="h0p", bufs=2) as hpool,
            tc.tile_pool(name="casc", bufs=2) as cascpool,
            tc.tile_pool(name="out", bufs=1) as opool,
            tc.tile_pool(name="ps", bufs=8, space="PSUM") as pspool,
        ):
            q_tile = cpool.tile([128, KC, b], fin)
            nc.sync.dma_start(out=q_tile, in_=q8.rearrange("(c p) m -> p c m", p=128))
            ones_t = cpool.tile([1, 1, 128], f8)
            nc.sync.dma_start(out=ones_t, in_=ones8[:, :, :])
            aug_t = cpool.tile([1, 1, ns], f8)
            nc.sync.dma_start(out=aug_t, in_=aug8[:, :, :])

            x8_r = x8.rearrange("(c p) n -> p c n", p=128)
            n_chunks = min(5, nt)
            tpc = (nt + n_chunks - 1) // n_chunks
            xj = []
            for j in range(0, nt, tpc):
                t0, t1 = j * CT, min((j + tpc) * CT, ns)
                xt = cpool.tile([128, KC, t1 - t0], fin, name=f"x{j}")
                nc.sync.dma_start(out=xt, in_=x8_r[:, :, t0:t1])
                xj.append((t0, xt))

            def xslice(t):
                for t0, xt in reversed(xj):
                    if t * CT >= t0:
                        return xt[:, :, t * CT - t0 : (t + 1) * CT - t0]
                raise AssertionError

            i8 = opool.tile([128, qt * TOP], mybir.dt.uint16)
            m8 = opool.tile([128, qt * TOP], f16)

            NEG = -30000.0
            for q in range(qt):
                h0 = hpool.tile([128, h0w], f16, name="h0")
                if h0w > ns:
                    nc.vector.memset(h0[:, ns:], NEG)
                qs = slice(q * 128, (q + 1) * 128)
                for t in range(nt):
                    ps = pspool.tile([128, CT], mybir.dt.float32)
                    xs = xslice(t)
                    for c in range(KC):
                        nc.tensor.matmul(
                            ps, lhsT=q_tile[:, c, qs], rhs=xs[:, c, :],
                            start=(c == 0), stop=False,
                        )
                    nc.tensor.matmul(
                        ps, lhsT=ones_t[:, 0, :],
                        rhs=aug_t[:, 0, t * CT : (t + 1) * CT],
                        start=False, stop=True,
                    )
                    nc.scalar.copy(h0[:, t * CT : (t + 1) * CT], ps)

                c1 = cascpool.tile([128, h0w // 2], f16, bufs=1)
                nc.vector.tensor_max(c1, h0[:, : h0w // 2], h0[:, h0w // 2 :])
                c2 = cascpool.tile([128, h0w // 4], f16)
                nc.vector.tensor_max(c2, c1[:, : h0w // 4], c1[:, h0w // 4 :])
                c3 = cascpool.tile([128, win], f16)
                nc.vector.tensor_max(c3, c2[:, :win], c2[:, win:])
                o = slice(q * TOP, (q + 1) * TOP)
                nc.vector.max(out=m8[:, o], in_=c3)
                nc.vector.max_index(out=i8[:, o], in_max=m8[:, o], in_values=c3)

            nc.sync.dma_start(
                out=idxs.rearrange("(q p) k -> p q k", p=128),
                in_=i8.rearrange("p (q k) -> p q k", q=qt),
            )
    nc.compile()
    return nc


def _get_nc():
    global _cached_nc
    if _cached_nc is None:
        _cached_nc = _build()
    return _cached_nc


def prepare_in_maps(X_tilde, X, w):
    """Device inputs for each core (bf16 q/x, fp8 aug). Shared with test.py."""
    F8 = ml_dtypes.float8_e4m3
    q8 = np.ascontiguousarray((2.0 * X_tilde).T).astype(FIN)  # [D, B]
    x_sq = np.einsum("nd,nd->n", X.astype(np.float64), X.astype(np.float64))
    aug = w[:, 0].astype(np.float64) - x_sq                   # [n]
    off = float(np.mean(aug))
    coarse = (aug - off).astype(np.float32).astype(F8)
    ones = np.ones((1, 1, 128), F8)

    in_maps = []
    for c in range(M):
        sl = slice(c * NS, (c + 1) * NS)
        in_maps.append(
            {
                "q8": q8,
                "x8": np.ascontiguousarray(X[sl].T).astype(FIN),
                "aug8": coarse[sl][None, None, :],
                "ones8": ones,
            }
        )
    return in_maps


def _expand_windows(idx):
    """[B, M, TOP] window ids -> candidate rows [B, M*TOP*WEXP] + validity."""
    win = _h0w(NS) // WEXP
    j = idx.astype(np.int64)[:, :, :, None] + win * np.arange(WEXP)[None, None, None, :]
    valid = j < NS                                            # [B,M,TOP,WEXP]
    rows = np.arange(M)[None, :, None, None] * NS + np.minimum(j, NS - 1)
    return rows.reshape(rows.shape[0], -1), valid.reshape(valid.shape[0], -1)


def _device_candidates(X_tilde, X, w):
    in_maps = prepare_in_maps(X_tilde, X, w)
    res = run_bass_kernel_spmd(_get_nc(), in_maps, core_ids=list(range(M)))
    idx = np.stack([res.results[c]["idxs"] for c in range(M)], axis=1)  # [B,M,8]
    return _expand_windows(idx)


def kernel(X_tilde, X, w, k):
    k = int(k)
    assert k <= 32, f"host merge sized for small k, got {k}"
    X_tilde = np.asarray(X_tilde, dtype=np.float32)
    X = np.asarray(X, dtype=np.float32)
    w = np.asarray(w, dtype=np.float32).reshape(N_TOTAL, 1)

    cand, valid = _device_candidates(X_tilde, X, w)          # [B, 512]

    # fp32 cut 512 -> 32 by the exact key, then fp64 exact top-k of those.
    x_sq = np.einsum("nd,nd->n", X, X)                       # [n] fp32
    q_sq = np.einsum("bd,bd->b", X_tilde, X_tilde)           # [B]
    Xc = X[cand]                                             # [B, 512, d]
    qx = np.einsum("bkd,bd->bk", Xc, X_tilde)                # [B, 512]
    d2 = q_sq[:, None] - 2.0 * qx + x_sq[cand]
    key = d2 - w[cand, 0]
    key[~valid] = np.inf
    margin = max(32, 2 * k)
    part = np.argpartition(key, margin, axis=1)[:, :margin]
    c32 = np.take_along_axis(cand, part, axis=1)             # [B, 32]

    Xc64 = X[c32].astype(np.float64)
    diff = Xc64 - X_tilde[:, None, :].astype(np.float64)
    d2e = np.einsum("bkd,bkd->bk", diff, diff)               # exact
    wc = w[c32, 0].astype(np.float64)
    key64 = d2e - wc
    sel = np.argpartition(key64, k, axis=1)[:, :k]           # exact k nearest
    d2k = np.take_along_axis(d2e, sel, axis=1)
    wk = np.take_along_axis(wc, sel, axis=1)
    act = wk - np.sqrt(d2k)                                  # K_COEF = 1.0
    return act.max(axis=1).astype(np.float32)


# revision 20
# speedup vs baseline: 3.4687x; 3.4687x over previous
"""NeighborDiscriminator kernel for 8x Trainium2 NeuronCores.

Math (reference): augmented-L2 kNN search, k=10, over n=100000 database rows,
B=1024 queries, d=512, followed by max over the k neighbors of
act_i = w_i - ||x_i - q||.

Selection key per (query q, candidate i):
    d2aug = ||q||^2 - 2 q.x_i + ||x_i||^2 + (max(w) - w_i)
Per-query-constant terms don't change the per-query ordering, so the device
ranks by  A = 2 q.x_i + (w_i - ||x_i||^2 - OFF)  (descending A == ascending
d2aug; OFF is a global constant centering the aug term).

Distribution: X / w sharded over 8 cores along n (12500 rows each), queries
replicated.

Device pipeline per core (bf16 was measured fastest on this part; fp8
DoubleRow and plain fp8 are both slower per matmul on real HW):
  - X^T resident in SBUF as bf16 (100KB/partition), queries bf16.
  - scores A = [128q x 500c] in PSUM: 4 plain bf16 matmuls + 1 bf16 rank-1
    that adds the centered aug row, all in one bf16 accumulation group
    (mixing an fp8 rank-1 into the group cost ~2.2ms on HW: per-instruction
    PE dtype switches are catastrophically slow).
  - ACT evicts PSUM -> SBUF fp16 into a per-query row h0[128, 12504]
    (a tensor_max reading both PSUM halves is illegal: one PSUM port).
  - DVE halves h0 3x with tensor_max (fp16 2-byte => 2x DVE mode) to 1563
    window maxima; one Max + MaxIndex gives the top-8 windows per query.
Each window covers 8 candidates: h0 columns i + 1563a (a<8) == shard rows.
Host expands the 8 winning windows (64 cand/core, 512/query) and re-ranks
exactly (fp32 cut to 32, fp64 top-k, then max_k(w - dist)).  The top-8
windows provably contain every top-8-by-device-score candidate of the core;
a true top-10 member is missed only if >=8 distinct same-core candidates
outscore it under ~1.5 abs device noise (P ~ 1e-8) or >=9 of the true
top-10 land in one core (P ~ 5e-7).  Verified end-to-end on the reference
seed by hostcheck.py.
"""

import numpy as np
import ml_dtypes

import concourse.bacc as bacc
import concourse.mybir as mybir
from concourse.tile import TileContext
from concourse.bass_utils import run_bass_kernel_spmd

B = 1024            # queries
N_TOTAL = 100000    # database rows
D = 512             # feature dim
M = 8               # cores
NS = N_TOTAL // M   # 12500 rows per core
CT = 500            # candidate tile width (PSUM bank = 512 fp32)
HCT = CT // 2       # 250: eviction pairs column s with s+250
KC = D // 128       # 4 contraction chunks
TOP = 8             # top-8 windows per query per core (DVE max8)
WEXP = 8            # candidates per window (1 evict-halve + 2 cascade halves)

FIN = ml_dtypes.bfloat16

_cached_nc = None


def _h0w(ns):
    """fp16 evicted-row width: ns padded to a multiple of WEXP."""
    return ((ns + WEXP - 1) // WEXP) * WEXP


def _build(b=B, ns=NS):
    qt = b // 128
    nt = ns // CT
    h0w = _h0w(ns)          # 12504 for ns=12500
    win = h0w // WEXP       # 1563

    nc = bacc.Bacc(
        "TRN2",
        target_bir_lowering=False,
        debug=False,
        enable_asserts=False,
        num_devices=M,
    )
    fin = mybir.dt.bfloat16
    f16 = mybir.dt.float16

    q8 = nc.dram_tensor("q8", [D, b], fin, kind="ExternalInput")
    x8 = nc.dram_tensor("x8", [D, ns], fin, kind="ExternalInput")
    aug8 = nc.dram_tensor("aug8", [1, 1, ns], fin, kind="ExternalInput")
    ones8 = nc.dram_tensor("ones8", [1, 1, 128], fin, kind="ExternalInput")
    idxs = nc.dram_tensor("idxs", [b, TOP], mybir.dt.uint16, kind="ExternalOutput")

    with TileContext(nc) as tc:
        with (
            tc.tile_pool(name="const", bufs=1) as cpool,
            tc.tile_pool(name="h0p", bufs=1) as hpool,
            tc.tile_pool(name="casc", bufs=1) as cascpool,
            tc.tile_pool(name="out", bufs=1) as opool,
            tc.tile_pool(name="ps", bufs=8, space="PSUM") as pspool,
        ):
            q_tile = cpool.tile([128, KC, b], fin)
            nc.sync.dma_start(out=q_tile, in_=q8.rearrange("(c p) m -> p c m", p=128))
            ones_t = cpool.tile([1, 1, 128], fin)
            nc.sync.dma_start(out=ones_t, in_=ones8[:, :, :])
            aug_t = cpool.tile([1, 1, ns], fin)
            nc.sync.dma_start(out=aug_t, in_=aug8[:, :, :])

            x8_r = x8.rearrange("(c p) n -> p c n", p=128)
            n_chunks = min(5, nt)
            tpc = (nt + n_chunks - 1) // n_chunks
            xj = []
            for j in range(0, nt, tpc):
                t0, t1 = j * CT, min((j + tpc) * CT, ns)
                xt = cpool.tile([128, KC, t1 - t0], fin, name=f"x{j}")
                nc.sync.dma_start(out=xt, in_=x8_r[:, :, t0:t1])
                xj.append((t0, xt))

            def xslice(t):
                for t0, xt in reversed(xj):
                    if t * CT >= t0:
                        return xt[:, :, t * CT - t0 : (t + 1) * CT - t0]
                raise AssertionError

            i8 = opool.tile([128, qt * TOP], mybir.dt.uint16)
            m8 = opool.tile([128, qt * TOP], f16)

            NEG = -30000.0
            for q in range(qt):
                h0 = hpool.tile([128, h0w], f16, name="h0")
                if h0w > ns:
                    nc.vector.memset(h0[:, ns:], NEG)
                qs = slice(q * 128, (q + 1) * 128)
                for t in range(nt):
                    ps = pspool.tile([128, CT], mybir.dt.float32)
                    xs = xslice(t)
                    for c in range(KC):
                        nc.tensor.matmul(
                            ps, lhsT=q_tile[:, c, qs], rhs=xs[:, c, :],
                            start=(c == 0), stop=False,
                        )
                    nc.tensor.matmul(
                        ps, lhsT=ones_t[:, 0, :],
                        rhs=aug_t[:, 0, t * CT : (t + 1) * CT],
                        start=False, stop=True,
                    )
                    nc.scalar.copy(h0[:, t * CT : (t + 1) * CT], ps)

                c1 = cascpool.tile([128, h0w // 2], f16, bufs=1)
                nc.vector.tensor_max(c1, h0[:, : h0w // 2], h0[:, h0w // 2 :])
                c2 = cascpool.tile([128, h0w // 4], f16)
                nc.vector.tensor_max(c2, c1[:, : h0w // 4], c1[:, h0w // 4 :])
                c3 = cascpool.tile([128, win], f16)
                nc.vector.tensor_max(c3, c2[:, :win], c2[:, win:])
                o = slice(q * TOP, (q + 1) * TOP)
                nc.vector.max(out=m8[:, o], in_=c3)
                nc.vector.max_index(out=i8[:, o], in_max=m8[:, o], in_values=c3)

            nc.sync.dma_start(
                out=idxs.rearrange("(q p) k -> p q k", p=128),
                in_=i8.rearrange("p (q k) -> p q k", q=qt),
            )
    nc.compile()
    return nc


def _get_nc():
    global _cached_nc
    if _cached_nc is None:
        _cached_nc = _build()
    return _cached_nc


def prepare_in_maps(X_tilde, X, w):
    """Device inputs for each core (bf16 q/x, fp8 aug). Shared with test.py."""
    q8 = np.ascontiguousarray((2.0 * X_tilde).T).astype(FIN)  # [D, B]
    x_sq = np.einsum("nd,nd->n", X.astype(np.float64), X.astype(np.float64))
    aug = w[:, 0].astype(np.float64) - x_sq                   # [n]
    off = float(np.mean(aug))
    coarse = (aug - off).astype(np.float32).astype(FIN)
    ones = np.ones((1, 1, 128), FIN)

    in_maps = []
    for c in range(M):
        sl = slice(c * NS, (c + 1) * NS)
        in_maps.append(
            {
                "q8": q8,
                "x8": np.ascontiguousarray(X[sl].T).astype(FIN),
                "aug8": coarse[sl][None, None, :],
                "ones8": ones,
            }
        )
    return in_maps


def _expand_windows(idx):
    """[B, M, TOP] window ids -> candidate rows [B, M*TOP*WEXP] + validity."""
    win = _h0w(NS) // WEXP
    j = idx.astype(np.int64)[:, :, :, None] + win * np.arange(WEXP)[None, None, None, :]
    valid = j < NS                                            # [B,M,TOP,WEXP]
    rows = np.arange(M)[None, :, None, None] * NS + np.minimum(j, NS - 1)
    return rows.reshape(rows.shape[0], -1), valid.reshape(valid.shape[0], -1)


def _device_candidates(X_tilde, X, w):
    in_maps = prepare_in_maps(X_tilde, X, w)
    res = run_bass_kernel_spmd(_get_nc(), in_maps, core_ids=list(range(M)))
    idx = np.stack([res.results[c]["idxs"] for c in range(M)], axis=1)  # [B,M,8]
    return _expand_windows(idx)


def kernel(X_tilde, X, w, k):
    k = int(k)
    assert k <= 32, f"host merge sized for small k, got {k}"
    X_tilde = np.asarray(X_tilde, dtype=np.float32)
    X = np.asarray(X, dtype=np.float32)
    w = np.asarray(w, dtype=np.float32).reshape(N_TOTAL, 1)

    cand, valid = _device_candidates(X_tilde, X, w)          # [B, 512]

    # fp32 cut 512 -> 32 by the exact key, then fp64 exact top-k of those.
    x_sq = np.einsum("nd,nd->n", X, X)                       # [n] fp32
    q_sq = np.einsum("bd,bd->b", X_tilde, X_tilde)           # [B]
    Xc = X[cand]                                             # [B, 512, d]
    qx = np.einsum("bkd,bd->bk", Xc, X_tilde)                # [B, 512]
    d2 = q_sq[:, None] - 2.0 * qx + x_sq[cand]
    key = d2 - w[cand, 0]
    key[~valid] = np.inf
    margin = max(32, 2 * k)
    part = np.argpartition(key, margin, axis=1)[:, :margin]
    c32 = np.take_along_axis(cand, part, axis=1)             # [B, 32]

    Xc64 = X[c32].astype(np.float64)
    diff = Xc64 - X_tilde[:, None, :].astype(np.float64)
    d2e = np.einsum("bkd,bkd->bk", diff, diff)               # exact
    wc = w[c32, 0].astype(np.float64)
    key64 = d2e - wc
    sel = np.argpartition(key64, k, axis=1)[:, :k]           # exact k nearest
    d2k = np.take_along_axis(d2e, sel, axis=1)
    wk = np.take_along_axis(wc, sel, axis=1)
    act = wk - np.sqrt(d2k)                                  # K_COEF = 1.0
    return act.max(axis=1).astype(np.float32)


# revision 22
# speedup vs baseline: 20.9429x; 6.0378x over previous
"""NeighborDiscriminator kernel for 8x Trainium2 NeuronCores.

Math (reference): augmented-L2 kNN search, k=10, over n=100000 database rows,
B=1024 queries, d=512, followed by max over the k neighbors of
act_i = w_i - ||x_i - q||.

Selection key per (query q, candidate i):
    d2aug = ||q||^2 - 2 q.x_i + ||x_i||^2 + (max(w) - w_i)
Per-query-constant terms don't change the per-query ordering, so the device
ranks by  A = 2 q.x_i + (w_i - ||x_i||^2 - OFF)  (descending A == ascending
d2aug; OFF is a global constant centering the aug term).

Distribution: X / w sharded over 8 cores along n (12500 rows each), queries
replicated.

Device pipeline per core (bf16 was measured fastest on this part; fp8
DoubleRow and plain fp8 are both slower per matmul on real HW):
  - X^T resident in SBUF as bf16 (100KB/partition), queries bf16.
  - scores A = [128q x 500c] in PSUM: 4 plain bf16 matmuls + 1 bf16 rank-1
    that adds the centered aug row, all in one bf16 accumulation group
    (mixing an fp8 rank-1 into the group cost ~2.2ms on HW: per-instruction
    PE dtype switches are catastrophically slow).
  - ACT evicts PSUM -> SBUF fp16 into a per-query row h0[128, 12504]
    (a tensor_max reading both PSUM halves is illegal: one PSUM port).
  - DVE halves h0 3x with tensor_max (fp16 2-byte => 2x DVE mode) to 1563
    window maxima; one Max + MaxIndex gives the top-8 windows per query.
Each window covers 8 candidates: h0 columns i + 1563a (a<8) == shard rows.
Host expands the 8 winning windows (64 cand/core, 512/query) and re-ranks
exactly (fp32 cut to 32, fp64 top-k, then max_k(w - dist)).  The top-8
windows provably contain every top-8-by-device-score candidate of the core;
a true top-10 member is missed only if >=8 distinct same-core candidates
outscore it under ~1.5 abs device noise (P ~ 1e-8) or >=9 of the true
top-10 land in one core (P ~ 5e-7).  Verified end-to-end on the reference
seed by hostcheck.py.
"""

import numpy as np
import ml_dtypes

import concourse.bacc as bacc
import concourse.mybir as mybir
from concourse.tile import TileContext
from concourse.bass_utils import run_bass_kernel_spmd

B = 1024            # queries
N_TOTAL = 100000    # database rows
D = 512             # feature dim
M = 8               # cores
NS = N_TOTAL // M   # 12500 rows per core
CT = 500            # candidate tile width (PSUM bank = 512 fp32)
HCT = CT // 2       # 250: eviction pairs column s with s+250
KC = D // 128       # 4 contraction chunks
TOP = 8             # top-8 windows per query per core (DVE max8)
WEXP = 8            # candidates per window (1 evict-halve + 2 cascade halves)

FIN = ml_dtypes.bfloat16

_cached_nc = None


def _h0w(ns):
    """fp16 evicted-row width: ns padded to a multiple of WEXP."""
    return ((ns + WEXP - 1) // WEXP) * WEXP


def _build(b=B, ns=NS, chain=1):
    """chain > 1 repeats the whole body (incl. input DMAs) in one NEFF so
    test.py can measure per-exec device time free of dispatch overhead."""
    qt = b // 128
    nt = ns // CT
    h0w = _h0w(ns)          # 12504 for ns=12500
    win = h0w // WEXP       # 1563

    nc = bacc.Bacc(
        "TRN2",
        target_bir_lowering=False,
        debug=False,
        enable_asserts=False,
        num_devices=M,
    )
    fin = mybir.dt.bfloat16
    f16 = mybir.dt.float16

    q8 = nc.dram_tensor("q8", [D, b], fin, kind="ExternalInput")
    x8 = nc.dram_tensor("x8", [D, ns], fin, kind="ExternalInput")
    aug8 = nc.dram_tensor("aug8", [1, 1, ns], fin, kind="ExternalInput")
    ones8 = nc.dram_tensor("ones8", [1, 1, 128], fin, kind="ExternalInput")
    idxs = nc.dram_tensor("idxs", [b, TOP], mybir.dt.uint16, kind="ExternalOutput")

    with TileContext(nc) as tc:
        with (
            tc.tile_pool(name="const", bufs=1) as cpool,
            tc.tile_pool(name="h0p", bufs=1) as hpool,
            tc.tile_pool(name="casc", bufs=1) as cascpool,
            tc.tile_pool(name="out", bufs=1) as opool,
            tc.tile_pool(name="ps", bufs=8, space="PSUM") as pspool,
        ):
            for _rep in range(chain):
                q_tile = cpool.tile([128, KC, b], fin, name="q_tile")
                nc.sync.dma_start(
                    out=q_tile, in_=q8.rearrange("(c p) m -> p c m", p=128)
                )
                ones_t = cpool.tile([1, 1, 128], fin, name="ones_t")
                nc.sync.dma_start(out=ones_t, in_=ones8[:, :, :])
                aug_t = cpool.tile([1, 1, ns], fin, name="aug_t")
                nc.sync.dma_start(out=aug_t, in_=aug8[:, :, :])

                x8_r = x8.rearrange("(c p) n -> p c n", p=128)
                n_chunks = min(5, nt)
                tpc = (nt + n_chunks - 1) // n_chunks
                xj = []
                for j in range(0, nt, tpc):
                    t0, t1 = j * CT, min((j + tpc) * CT, ns)
                    xt = cpool.tile([128, KC, t1 - t0], fin, name=f"x{j}")
                    nc.sync.dma_start(out=xt, in_=x8_r[:, :, t0:t1])
                    xj.append((t0, xt))

                def xslice(t):
                    for t0, xt in reversed(xj):
                        if t * CT >= t0:
                            return xt[:, :, t * CT - t0 : (t + 1) * CT - t0]
                    raise AssertionError

                i8 = opool.tile([128, qt * TOP], mybir.dt.uint16, name="i8")
                m8 = opool.tile([128, qt * TOP], f16, name="m8")

                NEG = -30000.0
                for q in range(qt):
                    h0 = hpool.tile([128, h0w], f16, name="h0")
                    if h0w > ns:
                        nc.vector.memset(h0[:, ns:], NEG)
                    qs = slice(q * 128, (q + 1) * 128)
                    for t in range(nt):
                        ps = pspool.tile([128, CT], mybir.dt.float32)
                        xs = xslice(t)
                        for c in range(KC):
                            nc.tensor.matmul(
                                ps, lhsT=q_tile[:, c, qs], rhs=xs[:, c, :],
                                start=(c == 0), stop=False,
                            )
                        nc.tensor.matmul(
                            ps, lhsT=ones_t[:, 0, :],
                            rhs=aug_t[:, 0, t * CT : (t + 1) * CT],
                            start=False, stop=True,
                        )
                        nc.scalar.copy(h0[:, t * CT : (t + 1) * CT], ps)

                    c1 = cascpool.tile([128, h0w // 2], f16, name="c1")
                    nc.vector.tensor_max(c1, h0[:, : h0w // 2], h0[:, h0w // 2 :])
                    c2 = cascpool.tile([128, h0w // 4], f16, name="c2")
                    nc.vector.tensor_max(c2, c1[:, : h0w // 4], c1[:, h0w // 4 :])
                    c3 = cascpool.tile([128, win], f16, name="c3")
                    nc.vector.tensor_max(c3, c2[:, :win], c2[:, win:])
                    o = slice(q * TOP, (q + 1) * TOP)
                    nc.vector.max(out=m8[:, o], in_=c3)
                    nc.vector.max_index(out=i8[:, o], in_max=m8[:, o], in_values=c3)

                nc.sync.dma_start(
                    out=idxs.rearrange("(q p) k -> p q k", p=128),
                    in_=i8.rearrange("p (q k) -> p q k", q=qt),
                )
    nc.compile()
    return nc


def _get_nc():
    global _cached_nc
    if _cached_nc is None:
        _cached_nc = _build()
    return _cached_nc


def prepare_in_maps(X_tilde, X, w):
    """Device inputs for each core (bf16 q/x, fp8 aug). Shared with test.py."""
    q8 = np.ascontiguousarray((2.0 * X_tilde).T).astype(FIN)  # [D, B]
    x_sq = np.einsum("nd,nd->n", X.astype(np.float64), X.astype(np.float64))
    aug = w[:, 0].astype(np.float64) - x_sq                   # [n]
    off = float(np.mean(aug))
    coarse = (aug - off).astype(np.float32).astype(FIN)
    ones = np.ones((1, 1, 128), FIN)

    in_maps = []
    for c in range(M):
        sl = slice(c * NS, (c + 1) * NS)
        in_maps.append(
            {
                "q8": q8,
                "x8": np.ascontiguousarray(X[sl].T).astype(FIN),
                "aug8": coarse[sl][None, None, :],
                "ones8": ones,
            }
        )
    return in_maps


def _expand_windows(idx):
    """[B, M, TOP] window ids -> candidate rows [B, M*TOP*WEXP] + validity."""
    win = _h0w(NS) // WEXP
    j = idx.astype(np.int64)[:, :, :, None] + win * np.arange(WEXP)[None, None, None, :]
    valid = j < NS                                            # [B,M,TOP,WEXP]
    rows = np.arange(M)[None, :, None, None] * NS + np.minimum(j, NS - 1)
    return rows.reshape(rows.shape[0], -1), valid.reshape(valid.shape[0], -1)


def _device_candidates(X_tilde, X, w):
    in_maps = prepare_in_maps(X_tilde, X, w)
    res = run_bass_kernel_spmd(_get_nc(), in_maps, core_ids=list(range(M)))
    idx = np.stack([res.results[c]["idxs"] for c in range(M)], axis=1)  # [B,M,8]
    return _expand_windows(idx)


def kernel(X_tilde, X, w, k):
    k = int(k)
    assert k <= 32, f"host merge sized for small k, got {k}"
    X_tilde = np.asarray(X_tilde, dtype=np.float32)
    X = np.asarray(X, dtype=np.float32)
    w = np.asarray(w, dtype=np.float32).reshape(N_TOTAL, 1)

    cand, valid = _device_candidates(X_tilde, X, w)          # [B, 512]

    # fp32 cut 512 -> 32 by the exact key, then fp64 exact top-k of those.
    x_sq = np.einsum("nd,nd->n", X, X)                       # [n] fp32
    q_sq = np.einsum("bd,bd->b", X_tilde, X_tilde)           # [B]
    Xc = X[cand]                                             # [B, 512, d]
    qx = np.einsum("bkd,bd->bk", Xc, X_tilde)                # [B, 512]
    d2 = q_sq[:, None] - 2.0 * qx + x_sq[cand]
    key = d2 - w[cand, 0]
    key[~valid] = np.inf
    margin = max(32, 2 * k)
    part = np.argpartition(key, margin, axis=1)[:, :margin]
    c32 = np.take_along_axis(cand, part, axis=1)             # [B, 32]

    Xc64 = X[c32].astype(np.float64)
    diff = Xc64 - X_tilde[:, None, :].astype(np.float64)
    d2e = np.einsum("bkd,bkd->bk", diff, diff)               # exact
    wc = w[c32, 0].astype(np.float64)
    key64 = d2e - wc
    sel = np.argpartition(key64, k, axis=1)[:, :k]           # exact k nearest
    d2k = np.take_along_axis(d2e, sel, axis=1)
    wk = np.take_along_axis(wc, sel, axis=1)
    act = wk - np.sqrt(d2k)                                  # K_COEF = 1.0
    return act.max(axis=1).astype(np.float32)


# revision 28
# speedup vs baseline: 47.1341x; 2.2506x over previous
"""NeighborDiscriminator kernel for 8x Trainium2 NeuronCores.

Math (reference): augmented-L2 kNN search, k=10, over n=100000 database rows,
B=1024 queries, d=512, followed by max over the k neighbors of
act_i = w_i - ||x_i - q||.

Selection key per (query q, candidate i):
    d2aug = ||q||^2 - 2 q.x_i + ||x_i||^2 + (max(w) - w_i)
Per-query-constant terms don't change the per-query ordering, so the device
ranks by  A = 2 q.x_i + (w_i - ||x_i||^2 - OFF)  (descending A == ascending
d2aug; OFF is a global constant centering the aug term).

Distribution: X / w sharded over 8 cores along n (12500 rows each), queries
replicated.

Device pipeline per core (bf16 was measured fastest on this part; fp8
DoubleRow and plain fp8 are both slower per matmul on real HW):
  - X^T resident in SBUF as bf16 (100KB/partition), queries bf16.
  - scores A = [128q x 500c] in PSUM: 4 plain bf16 matmuls + 1 bf16 rank-1
    that adds the centered aug row, all in one bf16 accumulation group
    (mixing an fp8 rank-1 into the group cost ~2.2ms on HW: per-instruction
    PE dtype switches are catastrophically slow).
  - ACT evicts PSUM -> SBUF fp16 into a per-query row h0[128, 12504]
    (a tensor_max reading both PSUM halves is illegal: one PSUM port).
  - DVE halves h0 3x with tensor_max (fp16 2-byte => 2x DVE mode) to 1563
    window maxima; one Max + MaxIndex gives the top-8 windows per query.
Each window covers 8 candidates: h0 columns i + 1563a (a<8) == shard rows.
Host expands the 8 winning windows (64 cand/core, 512/query) and re-ranks
exactly (fp32 cut to 32, fp64 top-k, then max_k(w - dist)).  The top-8
windows provably contain every top-8-by-device-score candidate of the core;
a true top-10 member is missed only if >=8 distinct same-core candidates
outscore it under ~1.5 abs device noise (P ~ 1e-8) or >=9 of the true
top-10 land in one core (P ~ 5e-7).  Verified end-to-end on the reference
seed by hostcheck.py.
"""

import numpy as np
import ml_dtypes

import concourse.bacc as bacc
import concourse.mybir as mybir
from concourse.tile import TileContext
from concourse.bass_utils import run_bass_kernel_spmd

B = 1024            # queries
N_TOTAL = 100000    # database rows
D = 512             # feature dim
M = 8               # cores
NS = N_TOTAL // M   # 12500 rows per core
CT = 500            # candidate tile width (PSUM bank = 512 fp32)
HCT = CT // 2       # 250: eviction pairs column s with s+250
KC = D // 128       # 4 contraction chunks
TOP = 8             # top-8 windows per query per core (DVE max8)
WEXP = 8            # candidates per window (1 evict-halve + 2 cascade halves)

FIN = ml_dtypes.bfloat16

_cached_nc = None


def _h0w(ns):
    """fp16 evicted-row width: ns padded to a multiple of WEXP."""
    return ((ns + WEXP - 1) // WEXP) * WEXP


def _build(b=B, ns=NS, chain=1):
    """chain > 1 repeats the whole body (incl. input DMAs) in one NEFF so
    test.py can measure per-exec device time free of dispatch overhead."""
    qt = b // 128
    nt = ns // CT
    h0w = _h0w(ns)          # 12504 for ns=12500
    win = h0w // WEXP       # 1563

    nc = bacc.Bacc(
        "TRN2",
        target_bir_lowering=False,
        debug=False,
        enable_asserts=False,
        num_devices=M,
    )
    fin = mybir.dt.bfloat16
    f16 = mybir.dt.float16

    q8 = nc.dram_tensor("q8", [D, b], fin, kind="ExternalInput")
    x8 = nc.dram_tensor("x8", [D, ns], fin, kind="ExternalInput")
    aug8 = nc.dram_tensor("aug8", [1, 1, ns], fin, kind="ExternalInput")
    ones8 = nc.dram_tensor("ones8", [1, 1, 128], fin, kind="ExternalInput")
    idxs = nc.dram_tensor("idxs", [b, TOP], mybir.dt.uint16, kind="ExternalOutput")

    with TileContext(nc) as tc:
        with (
            tc.tile_pool(name="const", bufs=1) as cpool,
            tc.tile_pool(name="h0p", bufs=1) as hpool,
            tc.tile_pool(name="casc", bufs=1) as cascpool,
            tc.tile_pool(name="out", bufs=1) as opool,
            tc.tile_pool(name="ps", bufs=8, space="PSUM") as pspool,
        ):
            for _rep in range(chain):
                q_tile = cpool.tile([128, KC, b], fin, name="q_tile")
                nc.sync.dma_start(
                    out=q_tile, in_=q8.rearrange("(c p) m -> p c m", p=128)
                )
                ones_t = cpool.tile([1, 1, 128], fin, name="ones_t")
                nc.sync.dma_start(out=ones_t, in_=ones8[:, :, :])
                aug_t = cpool.tile([1, 1, ns], fin, name="aug_t")
                nc.sync.dma_start(out=aug_t, in_=aug8[:, :, :])

                x8_r = x8.rearrange("(c p) n -> p c n", p=128)
                n_chunks = min(5, nt)
                tpc = (nt + n_chunks - 1) // n_chunks
                xj = []
                for j in range(0, nt, tpc):
                    t0, t1 = j * CT, min((j + tpc) * CT, ns)
                    xt = cpool.tile([128, KC, t1 - t0], fin, name=f"x{j}")
                    nc.sync.dma_start(out=xt, in_=x8_r[:, :, t0:t1])
                    xj.append((t0, xt))

                def xslice(t):
                    for t0, xt in reversed(xj):
                        if t * CT >= t0:
                            return xt[:, :, t * CT - t0 : (t + 1) * CT - t0]
                    raise AssertionError

                i8 = opool.tile([128, qt * TOP], mybir.dt.uint16, name="i8")
                m8 = opool.tile([128, qt * TOP], f16, name="m8")

                NEG = -30000.0
                for q in range(qt):
                    h0 = hpool.tile([128, h0w], f16, name="h0")
                    if h0w > ns:
                        nc.vector.memset(h0[:, ns:], NEG)
                    qs = slice(q * 128, (q + 1) * 128)
                    for t in range(nt):
                        ps = pspool.tile([128, CT], mybir.dt.float32)
                        xs = xslice(t)
                        for c in range(KC):
                            nc.tensor.matmul(
                                ps, lhsT=q_tile[:, c, qs], rhs=xs[:, c, :],
                                start=(c == 0), stop=False,
                            )
                        nc.tensor.matmul(
                            ps, lhsT=ones_t[:, 0, :],
                            rhs=aug_t[:, 0, t * CT : (t + 1) * CT],
                            start=False, stop=True,
                        )
                        # evictions split ACT/DVE (GPSIMD cannot read PSUM)
                        if t % 25 < 21:
                            nc.scalar.copy(h0[:, t * CT : (t + 1) * CT], ps)
                        else:
                            nc.vector.tensor_copy(
                                h0[:, t * CT : (t + 1) * CT], ps
                            )

                    c1 = cascpool.tile([128, h0w // 2], f16, name="c1")
                    nc.vector.tensor_max(c1, h0[:, : h0w // 2], h0[:, h0w // 2 :])
                    c2 = cascpool.tile([128, h0w // 4], f16, name="c2")
                    nc.vector.tensor_max(c2, c1[:, : h0w // 4], c1[:, h0w // 4 :])
                    c3 = cascpool.tile([128, win], f16, name="c3")
                    nc.vector.tensor_max(c3, c2[:, :win], c2[:, win:])
                    o = slice(q * TOP, (q + 1) * TOP)
                    nc.vector.max(out=m8[:, o], in_=c3)
                    nc.vector.max_index(out=i8[:, o], in_max=m8[:, o], in_values=c3)

                nc.sync.dma_start(
                    out=idxs.rearrange("(q p) k -> p q k", p=128),
                    in_=i8.rearrange("p (q k) -> p q k", q=qt),
                )
    nc.compile()
    return nc


def _get_nc():
    global _cached_nc
    if _cached_nc is None:
        _cached_nc = _build()
    return _cached_nc


def prepare_in_maps(X_tilde, X, w):
    """Device inputs for each core (bf16 q/x, fp8 aug). Shared with test.py."""
    q8 = np.ascontiguousarray((2.0 * X_tilde).T).astype(FIN)  # [D, B]
    x_sq = np.einsum("nd,nd->n", X.astype(np.float64), X.astype(np.float64))
    aug = w[:, 0].astype(np.float64) - x_sq                   # [n]
    off = float(np.mean(aug))
    coarse = (aug - off).astype(np.float32).astype(FIN)
    ones = np.ones((1, 1, 128), FIN)

    in_maps = []
    for c in range(M):
        sl = slice(c * NS, (c + 1) * NS)
        in_maps.append(
            {
                "q8": q8,
                "x8": np.ascontiguousarray(X[sl].T).astype(FIN),
                "aug8": coarse[sl][None, None, :],
                "ones8": ones,
            }
        )
    return in_maps


def _expand_windows(idx):
    """[B, M, TOP] window ids -> candidate rows [B, M*TOP*WEXP] + validity."""
    win = _h0w(NS) // WEXP
    j = idx.astype(np.int64)[:, :, :, None] + win * np.arange(WEXP)[None, None, None, :]
    valid = j < NS                                            # [B,M,TOP,WEXP]
    rows = np.arange(M)[None, :, None, None] * NS + np.minimum(j, NS - 1)
    return rows.reshape(rows.shape[0], -1), valid.reshape(valid.shape[0], -1)


def _device_candidates(X_tilde, X, w):
    in_maps = prepare_in_maps(X_tilde, X, w)
    res = run_bass_kernel_spmd(_get_nc(), in_maps, core_ids=list(range(M)))
    idx = np.stack([res.results[c]["idxs"] for c in range(M)], axis=1)  # [B,M,8]
    return _expand_windows(idx)


def kernel(X_tilde, X, w, k):
    k = int(k)
    assert k <= 32, f"host merge sized for small k, got {k}"
    X_tilde = np.asarray(X_tilde, dtype=np.float32)
    X = np.asarray(X, dtype=np.float32)
    w = np.asarray(w, dtype=np.float32).reshape(N_TOTAL, 1)

    cand, valid = _device_candidates(X_tilde, X, w)          # [B, 512]

    # fp32 cut 512 -> 32 by the exact key, then fp64 exact top-k of those.
    x_sq = np.einsum("nd,nd->n", X, X)                       # [n] fp32
    q_sq = np.einsum("bd,bd->b", X_tilde, X_tilde)           # [B]
    Xc = X[cand]                                             # [B, 512, d]
    qx = np.einsum("bkd,bd->bk", Xc, X_tilde)                # [B, 512]
    d2 = q_sq[:, None] - 2.0 * qx + x_sq[cand]
    key = d2 - w[cand, 0]
    key[~valid] = np.inf
    margin = max(32, 2 * k)
    part = np.argpartition(key, margin, axis=1)[:, :margin]
    c32 = np.take_along_axis(cand, part, axis=1)             # [B, 32]

    Xc64 = X[c32].astype(np.float64)
    diff = Xc64 - X_tilde[:, None, :].astype(np.float64)
    d2e = np.einsum("bkd,bkd->bk", diff, diff)               # exact
    wc = w[c32, 0].astype(np.float64)
    key64 = d2e - wc
    sel = np.argpartition(key64, k, axis=1)[:, :k]           # exact k nearest
    d2k = np.take_along_axis(d2e, sel, axis=1)
    wk = np.take_along_axis(wc, sel, axis=1)
    act = wk - np.sqrt(d2k)                                  # K_COEF = 1.0
    return act.max(axis=1).astype(np.float32)
